# revision 1
# baseline (speedup 1.0000x reference)
"""Trainium2 Bass kernel for nn_ClassifierModel (nms_detection).

Computation (reference):
    h    = relu(features @ conv_w + conv_b)        # (B,H,W,C)@(C,D) -> (B,H,W,D)
    flat = h.reshape(B, F)                         # F = H*W*D = 401408
    cls  = flat @ cls_w + cls_b                    # (B, 64)
    bbox = flat @ bbox_w + bbox_b                  # (B, 128)
    <tiny postprocessing with roi -> (B, P, 5)>

Sharding: the flatten (contraction) dim F is split across the 8 cores by
slicing H into 8 chunks of 28 rows. Each core computes its conv slice and a
partial (B, 192) product against its slice of [cls_w | bbox_w]; the host sums
the 8 partials and runs the tiny postprocessing. This reads each dense-weight
element exactly once across the machine (the weights dominate HBM traffic).

Per-core device layout (matmul operands bf16, accumulation fp32 in PSUM):
    featT  (4,128,NB) : features slice, transposed to (c, pix*B+b) columns, bf16
    convw  (4,128,256): conv_w k-tiles (c on partitions), bf16
    convb  (2,128,1)  : conv_b halves (d on partitions), fp32
    wmat   (128,NT*192): [cls|bbox] rows f-tiled in q-major consumption order
    out    (16,192)   : partial fp32 [cls|bbox] sums for this core's f range

Stage 1 produces h^T with d on partitions and (pix, b) on the free axis; a
128-partition f-tile of flat^T is then exactly hT[q][:, pix*16:(pix+1)*16],
so stage 2 needs no transposes at all. Stage 2 consumes f-tiles in q-major
order (all q=0 tiles, then q=1) so it can start as soon as hT[0] exists; the
host lays wmat out in the same order. The W stream is chunked with small tail
chunks so the final chunk's matmul tail is short.
"""

import numpy as np

B = 16
H, W, C = 224, 7, 512
D = 256
P = 32
NCORES = 8
HSH = H // NCORES          # 28 rows of H per core
PIX = HSH * W              # 196 pixels per core per batch
FLOC = PIX * D             # 50176 contraction elements per core
NB = PIX * B               # 3136 stage-1 moving columns
NT = FLOC // 128           # 392 f-tiles per core
NQ = NT // 2               # 196 f-tiles per d-half
NTILE = 448                # stage-1 moving tile (3136 = 7*448)
CHUNKS = [42] * 8 + [28, 14, 7, 4, 3]   # W-stream chunks (sum = 392)
STRIDE = 16.0

_STATE = {}


def _build_module(reps=1):
    import concourse.mybir as mybir
    import concourse.tile as tile
    from concourse import bacc

    f32 = mybir.dt.float32
    bf16 = mybir.dt.bfloat16
    nc = bacc.Bacc("TRN2", target_bir_lowering=False, debug=False)

    featT = nc.dram_tensor("featT", [4, 128, NB], bf16, kind="ExternalInput")
    convw = nc.dram_tensor("convw", [4, 128, D], bf16, kind="ExternalInput")
    convb = nc.dram_tensor("convb", [2, 128, 1], f32, kind="ExternalInput")
    wmat = nc.dram_tensor("wmat", [128, NT * 192], bf16, kind="ExternalInput")
    if reps == 1:
        out = nc.dram_tensor("out", [16, 192], f32, kind="ExternalOutput")
    else:
        out = nc.dram_tensor("out", [reps, 16, 192], f32, kind="ExternalOutput")

    NTI = NB // NTILE  # 7 stage-1 n-tiles

    with tile.TileContext(nc) as tc:
        with (
            tc.tile_pool(name="res", bufs=2 if reps > 1 else 1) as res,
            tc.tile_pool(name="win", bufs=5) as win,
            tc.tile_pool(name="ps1", bufs=7, space="PSUM") as ps1p,
            tc.tile_pool(name="ps2", bufs=1, space="PSUM") as ps2p,
        ):
            for rep in range(reps):
                xts = []
                for t in range(4):
                    xt = res.tile([128, NB], bf16, tag=f"xt{t}", name=f"xt{t}")
                    nc.sync.dma_start(xt[:], featT[t])
                    xts.append(xt)
                cws = []
                for t in range(4):
                    cw = res.tile([128, D], bf16, tag=f"cw{t}", name=f"cw{t}")
                    nc.sync.dma_start(cw[:], convw[t])
                    cws.append(cw)
                cbs = []
                for q in range(2):
                    cb = res.tile([128, 1], f32, tag=f"cb{q}", name=f"cb{q}")
                    nc.sync.dma_start(cb[:], convb[q])
                    cbs.append(cb)
                hts = [res.tile([128, NB], bf16, tag=f"ht{q}", name=f"ht{q}")
                       for q in range(2)]

                # Stage 1, k-outer so matmuls start as soon as xt[0] lands:
                # hT[q][:, n-tile] = relu(conv_w[:, q-half].T @ featT + b)
                for q in range(2):
                    pss = [ps1p.tile([128, NTILE], f32, tag="ps",
                                     name=f"ps{q}_{n}") for n in range(NTI)]
                    for k in range(4):
                        for n in range(NTI):
                            nc.tensor.matmul(
                                pss[n][:],
                                cws[k][:, q * 128:(q + 1) * 128],
                                xts[k][:, n * NTILE:(n + 1) * NTILE],
                                start=(k == 0),
                                stop=(k == 3),
                            )
                    for n in range(NTI):
                        nc.scalar.activation(
                            hts[q][:, n * NTILE:(n + 1) * NTILE],
                            pss[n][:],
                            mybir.ActivationFunctionType.Relu,
                            bias=cbs[q],
                        )

                # Stage 2: acc(16,192) += hT-slice(128,16).T @ W-tile(128,192)
                # q-major f-tile order; W stream chunked per CHUNKS.
                acc = ps2p.tile([16, 192], f32, tag="acc", name="acc")
                pos = 0
                for ch in CHUNKS:
                    wc = win.tile([128, ch * 192], bf16, tag="wc", name="wc")
                    nc.sync.dma_start(
                        wc[:], wmat[:, pos * 192:(pos + ch) * 192])
                    for t in range(ch):
                        p_ = pos + t
                        q, pix = (0, p_) if p_ < NQ else (1, p_ - NQ)
                        nc.tensor.matmul(
                            acc[:],
                            hts[q][:, pix * 16:(pix + 1) * 16],
                            wc[:, t * 192:(t + 1) * 192],
                            start=(p_ == 0),
                            stop=(p_ == NT - 1),
                        )
                    pos += ch

                ot = res.tile([16, 192], f32, tag="ot", name="ot")
                nc.vector.tensor_copy(ot[:], acc[:])
                nc.sync.dma_start(out[:] if reps == 1 else out[rep], ot[:])

    nc.compile()
    return nc


def _prep_inputs(features, conv_w, conv_b, cls_w, bbox_w):
    import ml_dtypes

    f32 = np.float32
    bf16 = ml_dtypes.bfloat16
    features = np.asarray(features, dtype=f32).astype(bf16)
    conv_w = np.asarray(conv_w, dtype=f32).astype(bf16)
    conv_b = np.ascontiguousarray(conv_b, dtype=f32)

    convw_t = np.ascontiguousarray(conv_w.reshape(4, 128, D))
    convb_t = conv_b.reshape(2, 128, 1)

    in_maps = []
    for i in range(NCORES):
        fi = features[:, i * HSH:(i + 1) * HSH, :, :].reshape(B, PIX, C)
        featT = np.ascontiguousarray(fi.transpose(2, 1, 0).reshape(C, NB))

        # wmat block t holds W rows for the t-th f-tile in q-major order:
        # t < NQ -> f-tile 2t (q=0), else f-tile 2(t-NQ)+1 (q=1).
        wl = np.empty((128, NT, 192), dtype=bf16)
        r0, r1 = i * FLOC, (i + 1) * FLOC
        cw3 = cls_w[r0:r1].astype(bf16).reshape(NT, 128, 64)
        bw3 = bbox_w[r0:r1].astype(bf16).reshape(NT, 128, 128)
        wl[:, :NQ, :64] = cw3[0::2].transpose(1, 0, 2)
        wl[:, NQ:, :64] = cw3[1::2].transpose(1, 0, 2)
        wl[:, :NQ, 64:] = bw3[0::2].transpose(1, 0, 2)
        wl[:, NQ:, 64:] = bw3[1::2].transpose(1, 0, 2)

        in_maps.append({
            "featT": featT.reshape(4, 128, NB),
            "convw": convw_t,
            "convb": convb_t,
            "wmat": wl.reshape(128, NT * 192),
        })
    return in_maps


def _run_device(in_maps, trace=False, **kw):
    from concourse.bass_utils import run_bass_kernel_spmd

    if "nc" not in _STATE:
        _STATE["nc"] = _build_module()
    nc = _STATE["nc"]
    return run_bass_kernel_spmd(
        nc, in_maps, core_ids=list(range(NCORES)), trace=trace, **kw
    )


def _postprocess(partial, roi, cls_b, bbox_b):
    f32 = np.float32
    cls = partial[:, :64].astype(f32) + cls_b.astype(f32)
    bbox = partial[:, 64:].astype(f32) + bbox_b.astype(f32)

    obj = 1.0 / (1.0 + np.exp(-(cls[:, P:] - cls[:, :P]), dtype=f32))
    bb = bbox.reshape(B, 4, P).transpose(0, 2, 1)
    roi_img = roi.astype(f32) * f32(STRIDE)
    x = roi_img[:, :, 0] - bb[:, :, 1] * roi_img[:, :, 3]
    y = roi_img[:, :, 1]
    w = roi_img[:, :, 2] * np.exp(np.clip(bb[:, :, 2], -10.0, 10.0), dtype=f32)
    hh = roi_img[:, :, 3] * np.exp(np.clip(bb[:, :, 3], -10.0, 10.0), dtype=f32)
    return np.stack([x, y, w, hh, obj], axis=-1).astype(f32)


def kernel(features, roi, conv_w, conv_b, cls_w, cls_b, bbox_w, bbox_b):
    in_maps = _prep_inputs(features, conv_w, conv_b, cls_w, bbox_w)
    res = _run_device(in_maps)
    partial = np.zeros((B, 192), dtype=np.float64)
    for r in res.results:
        partial += np.asarray(r["out"], dtype=np.float64)
    return _postprocess(partial.astype(np.float32), np.asarray(roi),
                        np.asarray(cls_b), np.asarray(bbox_b))



# revision 3
# speedup vs baseline: 2.1847x; 2.1847x over previous
"""Trainium2 Bass kernel for nn_ClassifierModel (nms_detection).

Computation (reference):
    h    = relu(features @ conv_w + conv_b)        # (B,H,W,C)@(C,D) -> (B,H,W,D)
    flat = h.reshape(B, F)                         # F = H*W*D = 401408
    cls  = flat @ cls_w + cls_b                    # (B, 64)
    bbox = flat @ bbox_w + bbox_b                  # (B, 128)
    <tiny postprocessing with roi -> (B, P, 5)>

The reference never uses bbox coordinate 0 (x is overwritten by the second
assignment) and objectness only depends on cls1-cls0, so the device only
computes 128 output columns: [cls1-cls0 | bb1 | bb2 | bb3].

Sharding: the flatten (contraction) dim F is split across the 8 cores by
slicing H into 8 chunks of 28 rows. Each core computes its conv slice and a
partial (B, 128) product against its slice of the fused weight matrix; the
host sums the 8 partials and runs the tiny postprocessing. This reads each
dense-weight element exactly once across the machine.

Everything streams in fp8 (the kernel is HBM-bound): features and conv_w in
e3m4 (4 mantissa bits), hT and the fused dense weights in e4m3 so stage 2 can
run MatmulPerfMode.DoubleRow (two 128-deep f-tiles per instruction at 0.5
cycles/row). Host-side scales (S_FEAT/S_H/S_W) keep all values inside the TRN
fp8 normal ranges (e4m3 max normal is +-240) and are divided out on the host.

Per-core device layout:
    featT  (4,128,NB)        : features slice, (c, pix*B+b) columns, e3m4 * S_FEAT
    convw  (128, 4*256)      : conv_w k-tiles (c on partitions), e3m4 * S_H/S_FEAT
    convb  (2,128,1)         : conv_b halves (d on partitions), fp32 * S_H
    wmat   (128,NPAIR,2,128) : fused W rows, DoubleRow pair-major, e4m3 * S_W
    out    (16,128)          : partial fp32 sums for this core's f range

Stage 1 produces h^T with d on partitions and (pix, b) on the free axis; a
128-partition f-tile of flat^T is exactly hT[q][:, pix, :], and a DoubleRow
pair (pix=2j, 2j+1) is the contiguous 32-column slice hT[q][:, 2j:2j+2, :].
Stage 2 consumes pairs in q-major order (all q=0 pairs, then q=1) so it can
start as soon as hT[0] exists; the host lays wmat out in the same order.
"""

import numpy as np

B = 16
H, W, C = 224, 7, 512
D = 256
P = 32
NCORES = 8
HSH = H // NCORES          # 28 rows of H per core
PIX = HSH * W              # 196 pixels per core per batch
FLOC = PIX * D             # 50176 contraction elements per core
NB = PIX * B               # 3136 stage-1 moving columns
NT = FLOC // 128           # 392 f-tiles per core
NPAIR = NT // 2            # 196 DoubleRow pairs per core
NQP = NPAIR // 2           # 98 pairs per d-half
KOUT = 128                 # device output columns [clsdiff|bb1|bb2|bb3]
NTILE = 448                # stage-1 moving tile (3136 = 7*448)
NTI = NB // NTILE          # 7 stage-1 n-tiles
PCHUNKS = [21] * 8 + [14, 7, 4, 2, 1]   # W-stream chunks in pairs (sum = 196)
STRIDE = 16.0
S_FEAT = 2.0               # features scale into e3m4
S_H = 32.0                 # hT scale into e4m3
S_W = 16384.0              # dense-weight scale into e4m3

_STATE = {}


def _build_module(reps=1):
    import concourse.mybir as mybir
    import concourse.tile as tile
    from concourse import bacc

    f32 = mybir.dt.float32
    f8e3 = mybir.dt.float8e3
    f8e4 = mybir.dt.float8e4
    nc = bacc.Bacc("TRN2", target_bir_lowering=False, debug=False)

    featT = nc.dram_tensor("featT", [4, 128, NB], f8e3, kind="ExternalInput")
    convw = nc.dram_tensor("convw", [128, 4 * D], f8e3, kind="ExternalInput")
    convb = nc.dram_tensor("convb", [2, 128, 1], f32, kind="ExternalInput")
    wmat = nc.dram_tensor("wmat", [128, NPAIR, 2, KOUT], f8e4,
                          kind="ExternalInput")
    if reps == 1:
        out = nc.dram_tensor("out", [16, KOUT], f32, kind="ExternalOutput")
    else:
        out = nc.dram_tensor("out", [reps, 16, KOUT], f32,
                             kind="ExternalOutput")

    with tile.TileContext(nc) as tc:
        with (
            tc.tile_pool(name="res", bufs=2 if reps > 1 else 1) as res,
            tc.tile_pool(name="win", bufs=5) as win,
            tc.tile_pool(name="ps1", bufs=7, space="PSUM") as ps1p,
            tc.tile_pool(name="ps2", bufs=1, space="PSUM") as ps2p,
        ):
            for rep in range(reps):
                cw = res.tile([128, 4 * D], f8e3, tag="cw", name="cw")
                nc.sync.dma_start(cw[:], convw[:])
                cbs = []
                for q in range(2):
                    cb = res.tile([128, 1], f32, tag=f"cb{q}", name=f"cb{q}")
                    nc.sync.dma_start(cb[:], convb[q])
                    cbs.append(cb)
                xts = []
                for t in range(4):
                    xt = res.tile([128, NB], f8e3, tag=f"xt{t}", name=f"xt{t}")
                    nc.sync.dma_start(xt[:], featT[t])
                    xts.append(xt)
                hts = [res.tile([128, PIX, B], f8e4, tag=f"ht{q}",
                                name=f"ht{q}") for q in range(2)]

                # Stage 1, k-outer so matmuls start as soon as xt[0] lands:
                # hT[q][:, n-tile] = relu(conv_w[:, q-half].T @ featT + b)
                for q in range(2):
                    pss = [ps1p.tile([128, NTILE], f32, tag="ps",
                                     name=f"ps{q}_{n}") for n in range(NTI)]
                    for k in range(4):
                        for n in range(NTI):
                            nc.tensor.matmul(
                                pss[n][:],
                                cw[:, k * D + q * 128:k * D + (q + 1) * 128],
                                xts[k][:, n * NTILE:(n + 1) * NTILE],
                                start=(k == 0),
                                stop=(k == 3),
                            )
                    for n in range(NTI):
                        nc.scalar.activation(
                            hts[q][:, n * HSH:(n + 1) * HSH, :],
                            pss[n][:],
                            mybir.ActivationFunctionType.Relu,
                            bias=cbs[q],
                        )

                # Stage 2: acc(16,128) += DoubleRow pair matmuls.
                # pair p: lhsT = hT[q][:, 2j:2j+2, :]  (128,2,16)
                #         rhs  = wc[:, t]              (128,2,128)
                acc = ps2p.tile([16, KOUT], f32, tag="acc", name="acc")
                pos = 0
                for ch in PCHUNKS:
                    wc = win.tile([128, ch, 2, KOUT], f8e4, tag="wc",
                                  name="wc")
                    nc.sync.dma_start(wc[:], wmat[:, pos:pos + ch])
                    for t in range(ch):
                        p_ = pos + t
                        q, jj = (0, p_) if p_ < NQP else (1, p_ - NQP)
                        nc.tensor.matmul(
                            acc[:],
                            hts[q][:, 2 * jj:2 * jj + 2, :],
                            wc[:, t],
                            start=(p_ == 0),
                            stop=(p_ == NPAIR - 1),
                            perf_mode=mybir.MatmulPerfMode.DoubleRow,
                        )
                    pos += ch

                ot = res.tile([16, KOUT], f32, tag="ot", name="ot")
                nc.vector.tensor_copy(ot[:], acc[:])
                nc.sync.dma_start(out[:] if reps == 1 else out[rep], ot[:])

    nc.compile()
    return nc


def _prep_inputs(features, conv_w, conv_b, cls_w, bbox_w):
    import ml_dtypes

    f32 = np.float32
    f8e3 = ml_dtypes.float8_e3m4
    f8e4 = ml_dtypes.float8_e4m3

    features = (np.asarray(features, dtype=f32) * f32(S_FEAT)).astype(f8e3)
    convw_dev = np.ascontiguousarray(
        (np.asarray(conv_w, dtype=f32) * f32(S_H / S_FEAT))
        .reshape(4, 128, D).transpose(1, 0, 2).reshape(128, 4 * D)
    ).astype(f8e3)
    convb_dev = (np.ascontiguousarray(conv_b, dtype=f32) * f32(S_H)) \
        .reshape(2, 128, 1)

    # Fused device weights: [cls1-cls0 | bb1 | bb2 | bb3]  (F, 128)
    cls_w = np.asarray(cls_w, dtype=f32)
    bbox_w = np.asarray(bbox_w, dtype=f32)
    wfull = np.concatenate(
        [cls_w[:, P:] - cls_w[:, :P], bbox_w[:, P:]], axis=1) * f32(S_W)
    np.clip(wfull, -240.0, 240.0, out=wfull)
    wfull = wfull.astype(f8e4)

    in_maps = []
    for i in range(NCORES):
        fi = features[:, i * HSH:(i + 1) * HSH, :, :].reshape(B, PIX, C)
        featT_i = np.ascontiguousarray(fi.transpose(2, 1, 0).reshape(C, NB))

        # wmat[dd, pair, i2, col]: pair = q*NQP + jj consumes f-tiles
        # (pix=2jj+i2, q); W row for (pix, q, dd) is f = pix*256 + q*128 + dd.
        wc5 = wfull[i * FLOC:(i + 1) * FLOC].reshape(NQP, 2, 2, 128, KOUT)
        wl = np.ascontiguousarray(wc5.transpose(3, 2, 0, 1, 4)) \
            .reshape(128, NPAIR, 2, KOUT)

        in_maps.append({
            "featT": featT_i.reshape(4, 128, NB),
            "convw": convw_dev,
            "convb": convb_dev,
            "wmat": wl,
        })
    return in_maps


def _run_device(in_maps, trace=False, **kw):
    from concourse.bass_utils import run_bass_kernel_spmd

    if "nc" not in _STATE:
        _STATE["nc"] = _build_module()
    nc = _STATE["nc"]
    return run_bass_kernel_spmd(
        nc, in_maps, core_ids=list(range(NCORES)), trace=trace, **kw
    )


def _postprocess(partial, roi, cls_b, bbox_b):
    f32 = np.float32
    part = partial.astype(f32) / f32(S_H * S_W)
    cls_b = np.asarray(cls_b, dtype=f32)
    bbox_b = np.asarray(bbox_b, dtype=f32)
    clsdiff = part[:, :P] + (cls_b[P:] - cls_b[:P])
    bb1 = part[:, P:2 * P] + bbox_b[P:2 * P]
    bb2 = part[:, 2 * P:3 * P] + bbox_b[2 * P:3 * P]
    bb3 = part[:, 3 * P:] + bbox_b[3 * P:]

    obj = 1.0 / (1.0 + np.exp(-clsdiff, dtype=f32))
    roi_img = roi.astype(f32) * f32(STRIDE)
    x = roi_img[:, :, 0] - bb1 * roi_img[:, :, 3]
    y = roi_img[:, :, 1]
    w = roi_img[:, :, 2] * np.exp(np.clip(bb2, -10.0, 10.0), dtype=f32)
    hh = roi_img[:, :, 3] * np.exp(np.clip(bb3, -10.0, 10.0), dtype=f32)
    return np.stack([x, y, w, hh, obj], axis=-1).astype(f32)


def kernel(features, roi, conv_w, conv_b, cls_w, cls_b, bbox_w, bbox_b):
    in_maps = _prep_inputs(features, conv_w, conv_b, cls_w, bbox_w)
    res = _run_device(in_maps)
    partial = np.zeros((B, KOUT), dtype=np.float64)
    for r in res.results:
        partial += np.asarray(r["out"], dtype=np.float64)
    return _postprocess(partial.astype(np.float32), np.asarray(roi),
                        np.asarray(cls_b), np.asarray(bbox_b))


# revision 17
# speedup vs baseline: 2.3747x; 1.0870x over previous
"""Trainium2 Bass kernel for nn_ClassifierModel (nms_detection).

Computation (reference):
    h    = relu(features @ conv_w + conv_b)        # (B,H,W,C)@(C,D) -> (B,H,W,D)
    flat = h.reshape(B, F)                         # F = H*W*D = 401408
    cls  = flat @ cls_w + cls_b                    # (B, 64)
    bbox = flat @ bbox_w + bbox_b                  # (B, 128)
    <tiny postprocessing with roi -> (B, P, 5)>

The reference never uses bbox coordinate 0 (x is overwritten by the second
assignment) and objectness only depends on cls1-cls0, so the device only
computes 128 output columns: [cls1-cls0 | bb1 | bb2 | bb3].

Sharding: the flatten (contraction) dim F is split across the 8 cores by
slicing H into 8 chunks of 28 rows. Each core computes its conv slice and a
partial (B, 128) product against its slice of the fused weight matrix; the
host sums the 8 partials and runs the tiny postprocessing. This reads each
dense-weight element exactly once across the machine.

Everything streams in fp8 (the kernel is HBM-bound): features and conv_w in
e3m4 (4 mantissa bits), hT and the fused dense weights in e4m3 so stage 2 can
run MatmulPerfMode.DoubleRow (two 128-deep f-tiles per instruction at 0.5
cycles/row). Host-side scales (S_FEAT/S_H/S_W) keep all values inside the TRN
fp8 normal ranges (e4m3 max normal is +-240) and are divided out on the host.

Per-core device layout:
    featT  (4,128,NB)        : features slice, (c, pix*B+b) columns, e3m4 * S_FEAT
    convw  (128, 4*256)      : conv_w k-tiles (c on partitions), e3m4 * S_H/S_FEAT
    convb  (2,128,1)         : conv_b halves (d on partitions), fp32 * S_H
    wmat   (128,NPAIR,2,128) : fused W rows, DoubleRow pair-major, e4m3 * S_W
    out    (16,128)          : partial fp32 sums for this core's f range

Stage 1 produces h^T with d on partitions and (pix, b) on the free axis; a
128-partition f-tile of flat^T is exactly hT[q][:, pix, :], and a DoubleRow
pair (pix=2j, 2j+1) is the contiguous 32-column slice hT[q][:, 2j:2j+2, :].
Stage 2 consumes pairs in q-major order (all q=0 pairs, then q=1) so it can
start as soon as hT[0] exists; the host lays wmat out in the same order.
"""

import numpy as np

B = 16
H, W, C = 224, 7, 512
D = 256
P = 32
NCORES = 8
HSH = H // NCORES          # 28 rows of H per core
PIX = HSH * W              # 196 pixels per core per batch
FLOC = PIX * D             # 50176 contraction elements per core
NB = PIX * B               # 3136 stage-1 moving columns
NT = FLOC // 128           # 392 f-tiles per core
NPAIR = NT // 2            # 196 DoubleRow pairs per core
NQP = NPAIR // 2           # 98 pairs per d-half
KOUT = 128                 # device output columns [clsdiff|bb1|bb2|bb3]
NTILE = 448                # stage-1 moving tile (3136 = 7*448)
NTI = NB // NTILE          # 7 stage-1 n-tiles
PCHUNKS = [21] * 8 + [14, 7, 4, 2, 1]   # W-stream chunks in pairs (sum = 196)
STRIDE = 16.0
S_FEAT = 2.0               # features scale into e3m4
S_H = 32.0                 # hT scale into e4m3
S_W = 16384.0              # dense-weight scale into e4m3

_STATE = {}


def _build_module(reps=1):
    import concourse.mybir as mybir
    import concourse.tile as tile
    from concourse import bacc

    f32 = mybir.dt.float32
    f8e3 = mybir.dt.float8e3
    f8e4 = mybir.dt.float8e4
    nc = bacc.Bacc("TRN2", target_bir_lowering=False, debug=False)

    featT = nc.dram_tensor("featT", [4, 128, NB], f8e3, kind="ExternalInput")
    convw = nc.dram_tensor("convw", [128, 4 * D], f8e3, kind="ExternalInput")
    convb = nc.dram_tensor("convb", [2, 128, 1], f32, kind="ExternalInput")
    wmat = nc.dram_tensor("wmat", [128, NPAIR, 2, KOUT], f8e4,
                          kind="ExternalInput")
    if reps == 1:
        out = nc.dram_tensor("out", [16, KOUT], f32, kind="ExternalOutput")
    else:
        out = nc.dram_tensor("out", [reps, 16, KOUT], f32,
                             kind="ExternalOutput")

    with tile.TileContext(nc) as tc:
        with (
            tc.tile_pool(name="res", bufs=2 if reps > 1 else 1) as res,
            tc.tile_pool(name="win", bufs=8) as win,
            tc.tile_pool(name="ps1", bufs=7, space="PSUM") as ps1p,
            tc.tile_pool(name="ps2", bufs=1, space="PSUM") as ps2p,
        ):
            for rep in range(reps):
                # DMA order: conv_w first — its Ldweights issues ~1.4us before
                # the first matmul and anchors the PE p-state ramp (measured:
                # any ordering that delays the first PE instruction leaves
                # stage 1 at the MID/LOW p-state, which finishes hts[1] late
                # and stalls the W stream on the win-pool window). Then the
                # four big featT transfers (HWDGE gen pipelines behind them),
                # tiny biases last (needed only by the activations ~7us in).
                cw = res.tile([128, 4 * D], f8e3, tag="cw", name="cw")
                nc.sync.dma_start(cw[:], convw[:])
                xts = []
                for t in range(4):
                    xt = res.tile([128, NB], f8e3, tag=f"xt{t}", name=f"xt{t}")
                    nc.sync.dma_start(xt[:], featT[t])
                    xts.append(xt)
                cbs = []
                for q in range(2):
                    cb = res.tile([128, 1], f32, tag=f"cb{q}", name=f"cb{q}")
                    nc.sync.dma_start(cb[:], convb[q])
                    cbs.append(cb)
                hts = [res.tile([128, PIX, B], f8e4, tag=f"ht{q}",
                                name=f"ht{q}") for q in range(2)]

                # Stage 1, k-outer so matmuls start as soon as xt[0] lands:
                # hT[q][:, n-tile] = relu(conv_w[:, q-half].T @ featT + b)
                for q in range(2):
                    pss = [ps1p.tile([128, NTILE], f32, tag="ps",
                                     name=f"ps{q}_{n}") for n in range(NTI)]
                    for k in range(4):
                        for n in range(NTI):
                            nc.tensor.matmul(
                                pss[n][:],
                                cw[:, k * D + q * 128:k * D + (q + 1) * 128],
                                xts[k][:, n * NTILE:(n + 1) * NTILE],
                                start=(k == 0),
                                stop=(k == 3),
                            )
                    for n in range(NTI):
                        nc.scalar.activation(
                            hts[q][:, n * HSH:(n + 1) * HSH, :],
                            pss[n][:],
                            mybir.ActivationFunctionType.Relu,
                            bias=cbs[q],
                        )

                # Stage 2: acc(16,128) += DoubleRow pair matmuls.
                # pair p: lhsT = hT[q][:, 2j:2j+2, :]  (128,2,16)
                #         rhs  = wc[:, t]              (128,2,128)
                acc = ps2p.tile([16, KOUT], f32, tag="acc", name="acc")
                pos = 0
                for ch in PCHUNKS:
                    wc = win.tile([128, ch, 2, KOUT], f8e4, tag="wc",
                                  name="wc")
                    nc.sync.dma_start(wc[:], wmat[:, pos:pos + ch])
                    for t in range(ch):
                        p_ = pos + t
                        q, jj = (0, p_) if p_ < NQP else (1, p_ - NQP)
                        nc.tensor.matmul(
                            acc[:],
                            hts[q][:, 2 * jj:2 * jj + 2, :],
                            wc[:, t],
                            start=(p_ == 0),
                            stop=(p_ == NPAIR - 1),
                            perf_mode=mybir.MatmulPerfMode.DoubleRow,
                        )
                    pos += ch

                ot = res.tile([16, KOUT], f32, tag="ot", name="ot")
                nc.vector.tensor_copy(ot[:], acc[:])
                nc.sync.dma_start(out[:] if reps == 1 else out[rep], ot[:])

    nc.compile()
    return nc


def _prep_inputs(features, conv_w, conv_b, cls_w, bbox_w):
    import ml_dtypes

    f32 = np.float32
    f8e3 = ml_dtypes.float8_e3m4
    f8e4 = ml_dtypes.float8_e4m3

    features = (np.asarray(features, dtype=f32) * f32(S_FEAT)).astype(f8e3)
    convw_dev = np.ascontiguousarray(
        (np.asarray(conv_w, dtype=f32) * f32(S_H / S_FEAT))
        .reshape(4, 128, D).transpose(1, 0, 2).reshape(128, 4 * D)
    ).astype(f8e3)
    convb_dev = (np.ascontiguousarray(conv_b, dtype=f32) * f32(S_H)) \
        .reshape(2, 128, 1)

    # Fused device weights: [cls1-cls0 | bb1 | bb2 | bb3]  (F, 128)
    cls_w = np.asarray(cls_w, dtype=f32)
    bbox_w = np.asarray(bbox_w, dtype=f32)
    wfull = np.concatenate(
        [cls_w[:, P:] - cls_w[:, :P], bbox_w[:, P:]], axis=1) * f32(S_W)
    np.clip(wfull, -240.0, 240.0, out=wfull)
    wfull = wfull.astype(f8e4)

    in_maps = []
    for i in range(NCORES):
        fi = features[:, i * HSH:(i + 1) * HSH, :, :].reshape(B, PIX, C)
        featT_i = np.ascontiguousarray(fi.transpose(2, 1, 0).reshape(C, NB))

        # wmat[dd, pair, i2, col]: pair = q*NQP + jj consumes f-tiles
        # (pix=2jj+i2, q); W row for (pix, q, dd) is f = pix*256 + q*128 + dd.
        wc5 = wfull[i * FLOC:(i + 1) * FLOC].reshape(NQP, 2, 2, 128, KOUT)
        wl = np.ascontiguousarray(wc5.transpose(3, 2, 0, 1, 4)) \
            .reshape(128, NPAIR, 2, KOUT)

        in_maps.append({
            "featT": featT_i.reshape(4, 128, NB),
            "convw": convw_dev,
            "convb": convb_dev,
            "wmat": wl,
        })
    return in_maps


def _run_device(in_maps, trace=False, **kw):
    from concourse.bass_utils import run_bass_kernel_spmd

    if "nc" not in _STATE:
        _STATE["nc"] = _build_module()
    nc = _STATE["nc"]
    return run_bass_kernel_spmd(
        nc, in_maps, core_ids=list(range(NCORES)), trace=trace, **kw
    )


def _postprocess(partial, roi, cls_b, bbox_b):
    f32 = np.float32
    part = partial.astype(f32) / f32(S_H * S_W)
    cls_b = np.asarray(cls_b, dtype=f32)
    bbox_b = np.asarray(bbox_b, dtype=f32)
    clsdiff = part[:, :P] + (cls_b[P:] - cls_b[:P])
    bb1 = part[:, P:2 * P] + bbox_b[P:2 * P]
    bb2 = part[:, 2 * P:3 * P] + bbox_b[2 * P:3 * P]
    bb3 = part[:, 3 * P:] + bbox_b[3 * P:]

    obj = 1.0 / (1.0 + np.exp(-clsdiff, dtype=f32))
    roi_img = roi.astype(f32) * f32(STRIDE)
    x = roi_img[:, :, 0] - bb1 * roi_img[:, :, 3]
    y = roi_img[:, :, 1]
    w = roi_img[:, :, 2] * np.exp(np.clip(bb2, -10.0, 10.0), dtype=f32)
    hh = roi_img[:, :, 3] * np.exp(np.clip(bb3, -10.0, 10.0), dtype=f32)
    return np.stack([x, y, w, hh, obj], axis=-1).astype(f32)


def kernel(features, roi, conv_w, conv_b, cls_w, cls_b, bbox_w, bbox_b):
    in_maps = _prep_inputs(features, conv_w, conv_b, cls_w, bbox_w)
    res = _run_device(in_maps)
    partial = np.zeros((B, KOUT), dtype=np.float64)
    for r in res.results:
        partial += np.asarray(r["out"], dtype=np.float64)
    return _postprocess(partial.astype(np.float32), np.asarray(roi),
                        np.asarray(cls_b), np.asarray(bbox_b))


# revision 21
# speedup vs baseline: 2.3979x; 1.0098x over previous
"""Trainium2 Bass kernel for nn_ClassifierModel (nms_detection).

Computation (reference):
    h    = relu(features @ conv_w + conv_b)        # (B,H,W,C)@(C,D) -> (B,H,W,D)
    flat = h.reshape(B, F)                         # F = H*W*D = 401408
    cls  = flat @ cls_w + cls_b                    # (B, 64)
    bbox = flat @ bbox_w + bbox_b                  # (B, 128)
    <tiny postprocessing with roi -> (B, P, 5)>

The reference never uses bbox coordinate 0 (x is overwritten by the second
assignment) and objectness only depends on cls1-cls0, so the device only
computes 128 output columns: [cls1-cls0 | bb1 | bb2 | bb3].

Sharding: the flatten (contraction) dim F is split across the 8 cores by
slicing H into 8 chunks of 28 rows. Each core computes its conv slice and a
partial (B, 128) product against its slice of the fused weight matrix; the
host sums the 8 partials and runs the tiny postprocessing. This reads each
dense-weight element exactly once across the machine.

Everything streams in fp8 (the kernel is HBM-bound): features and conv_w in
e3m4 (4 mantissa bits), hT and the fused dense weights in e4m3 so stage 2 can
run MatmulPerfMode.DoubleRow (two 128-deep f-tiles per instruction at 0.5
cycles/row). Host-side scales (S_FEAT/S_H/S_W) keep all values inside the TRN
fp8 normal ranges (e4m3 max normal is +-240) and are divided out on the host.

Per-core device layout:
    featT  (4,128,NB)        : features slice, (c, pix*B+b) columns, e3m4 * S_FEAT
    convw  (128, 4*256)      : conv_w k-tiles (c on partitions), e3m4 * S_H/S_FEAT
    convb  (2,128,1)         : conv_b halves (d on partitions), fp32 * S_H
    wmat   (128,NPAIR,2,128) : fused W rows, DoubleRow pair-major, e4m3 * S_W
    out    (16,128)          : partial fp32 sums for this core's f range

Stage 1 produces h^T with d on partitions and (pix, b) on the free axis; a
128-partition f-tile of flat^T is exactly hT[q][:, pix, :], and a DoubleRow
pair (pix=2j, 2j+1) is the contiguous 32-column slice hT[q][:, 2j:2j+2, :].
Stage 2 consumes pairs in q-major order (all q=0 pairs, then q=1) so it can
start as soon as hT[0] exists; the host lays wmat out in the same order.
"""

import numpy as np

B = 16
H, W, C = 224, 7, 512
D = 256
P = 32
NCORES = 8
HSH = H // NCORES          # 28 rows of H per core
PIX = HSH * W              # 196 pixels per core per batch
FLOC = PIX * D             # 50176 contraction elements per core
NB = PIX * B               # 3136 stage-1 moving columns
NT = FLOC // 128           # 392 f-tiles per core
NPAIR = NT // 2            # 196 DoubleRow pairs per core
NQP = NPAIR // 2           # 98 pairs per d-half
KOUT = 128                 # device output columns [clsdiff|bb1|bb2|bb3]
NTILE = 448                # stage-1 moving tile (3136 = 7*448)
NTI = NB // NTILE          # 7 stage-1 n-tiles
PCHUNKS = [21] * 8 + [14, 7, 4, 2, 1]   # W-stream chunks in pairs (sum = 196)
STRIDE = 16.0
S_FEAT = 2.0               # features scale into e3m4
S_H = 32.0                 # hT scale into e4m3
S_W = 16384.0              # dense-weight scale into e4m3

_STATE = {}


def _build_module(reps=1):
    import concourse.mybir as mybir
    import concourse.tile as tile
    from concourse import bacc

    f32 = mybir.dt.float32
    f8e3 = mybir.dt.float8e3
    f8e4 = mybir.dt.float8e4
    nc = bacc.Bacc("TRN2", target_bir_lowering=False, debug=False)

    # Head split: fxh = conv_w + first 1024 cols of featT[0] (728ns transfer
    # hides the next DMA's HWDGE+DGE issue latency -> no stream bubble);
    # fxt = remaining 2112 cols of featT[0].
    fxh = nc.dram_tensor("fxh", [128, 4 * D + 1024], f8e3,
                         kind="ExternalInput")
    fxt = nc.dram_tensor("fxt", [128, NB - 1024], f8e3, kind="ExternalInput")
    featT = nc.dram_tensor("featT", [3, 128, NB], f8e3, kind="ExternalInput")
    convb = nc.dram_tensor("convb", [2, 128, 1], f32, kind="ExternalInput")
    wmat = nc.dram_tensor("wmat", [128, NPAIR, 2, KOUT], f8e4,
                          kind="ExternalInput")
    if reps == 1:
        out = nc.dram_tensor("out", [16, KOUT], f32, kind="ExternalOutput")
    else:
        out = nc.dram_tensor("out", [reps, 16, KOUT], f32,
                             kind="ExternalOutput")

    with tile.TileContext(nc) as tc:
        with (
            tc.tile_pool(name="res", bufs=2 if reps > 1 else 1) as res,
            tc.tile_pool(name="win", bufs=8) as win,
            tc.tile_pool(name="ps1", bufs=7, space="PSUM") as ps1p,
            tc.tile_pool(name="ps2", bufs=1, space="PSUM") as ps2p,
        ):
            for rep in range(reps):
                # DMA order: conv_w first — its Ldweights issues ~1.4us before
                # the first matmul and anchors the PE p-state ramp (measured:
                # any ordering that delays the first PE instruction leaves
                # stage 1 at the MID/LOW p-state, which finishes hts[1] late
                # and stalls the W stream on the win-pool window). Then the
                # four big featT transfers (HWDGE gen pipelines behind them),
                # tiny biases last (needed only by the activations ~7us in).
                big = res.tile([128, 4 * D + NB], f8e3, tag="big", name="big")
                nc.sync.dma_start(big[:, :4 * D + 1024], fxh[:])
                nc.sync.dma_start(big[:, 4 * D + 1024:], fxt[:])
                cw = big[:, :4 * D]
                xts = [big[:, 4 * D:]]
                for t in range(3):
                    xt = res.tile([128, NB], f8e3, tag=f"xt{t}", name=f"xt{t}")
                    nc.sync.dma_start(xt[:], featT[t])
                    xts.append(xt)
                cbs = []
                for q in range(2):
                    cb = res.tile([128, 1], f32, tag=f"cb{q}", name=f"cb{q}")
                    nc.sync.dma_start(cb[:], convb[q])
                    cbs.append(cb)
                hts = [res.tile([128, PIX, B], f8e4, tag=f"ht{q}",
                                name=f"ht{q}") for q in range(2)]

                # Stage 1, k-outer so matmuls start as soon as xt[0] lands:
                # hT[q][:, n-tile] = relu(conv_w[:, q-half].T @ featT + b)
                for q in range(2):
                    pss = [ps1p.tile([128, NTILE], f32, tag="ps",
                                     name=f"ps{q}_{n}") for n in range(NTI)]
                    # k=0 runs n=6 first: its first matmul depends on the fxt
                    # DMA, landing ~750ns after the Ldweights (cw via fxh) so
                    # the PE p-state ramp is anchored early.
                    for k in range(4):
                        for n in ([6, 0, 1, 2, 3, 4, 5] if k == 0
                                  else range(NTI)):
                            nc.tensor.matmul(
                                pss[n][:],
                                cw[:, k * D + q * 128:k * D + (q + 1) * 128],
                                xts[k][:, n * NTILE:(n + 1) * NTILE],
                                start=(k == 0),
                                stop=(k == 3),
                            )
                    for n in range(NTI):
                        nc.scalar.activation(
                            hts[q][:, n * HSH:(n + 1) * HSH, :],
                            pss[n][:],
                            mybir.ActivationFunctionType.Relu,
                            bias=cbs[q],
                        )

                # Stage 2: acc(16,128) += DoubleRow pair matmuls.
                # pair p: lhsT = hT[q][:, 2j:2j+2, :]  (128,2,16)
                #         rhs  = wc[:, t]              (128,2,128)
                acc = ps2p.tile([16, KOUT], f32, tag="acc", name="acc")
                pos = 0
                for ch in PCHUNKS:
                    wc = win.tile([128, ch, 2, KOUT], f8e4, tag="wc",
                                  name="wc")
                    nc.sync.dma_start(wc[:], wmat[:, pos:pos + ch])
                    for t in range(ch):
                        p_ = pos + t
                        q, jj = (0, p_) if p_ < NQP else (1, p_ - NQP)
                        nc.tensor.matmul(
                            acc[:],
                            hts[q][:, 2 * jj:2 * jj + 2, :],
                            wc[:, t],
                            start=(p_ == 0),
                            stop=(p_ == NPAIR - 1),
                            perf_mode=mybir.MatmulPerfMode.DoubleRow,
                        )
                    pos += ch

                ot = res.tile([16, KOUT], f32, tag="ot", name="ot")
                nc.vector.tensor_copy(ot[:], acc[:])
                nc.sync.dma_start(out[:] if reps == 1 else out[rep], ot[:])

    nc.compile()
    return nc


def _prep_inputs(features, conv_w, conv_b, cls_w, bbox_w):
    import ml_dtypes

    f32 = np.float32
    f8e3 = ml_dtypes.float8_e3m4
    f8e4 = ml_dtypes.float8_e4m3

    features = (np.asarray(features, dtype=f32) * f32(S_FEAT)).astype(f8e3)
    convw_dev = np.ascontiguousarray(
        (np.asarray(conv_w, dtype=f32) * f32(S_H / S_FEAT))
        .reshape(4, 128, D).transpose(1, 0, 2).reshape(128, 4 * D)
    ).astype(f8e3)
    convb_dev = (np.ascontiguousarray(conv_b, dtype=f32) * f32(S_H)) \
        .reshape(2, 128, 1)

    # Fused device weights: [cls1-cls0 | bb1 | bb2 | bb3]  (F, 128)
    cls_w = np.asarray(cls_w, dtype=f32)
    bbox_w = np.asarray(bbox_w, dtype=f32)
    wfull = np.concatenate(
        [cls_w[:, P:] - cls_w[:, :P], bbox_w[:, P:]], axis=1) * f32(S_W)
    np.clip(wfull, -240.0, 240.0, out=wfull)
    wfull = wfull.astype(f8e4)

    in_maps = []
    for i in range(NCORES):
        fi = features[:, i * HSH:(i + 1) * HSH, :, :].reshape(B, PIX, C)
        featT_i = np.ascontiguousarray(fi.transpose(2, 1, 0).reshape(C, NB))

        # wmat[dd, pair, i2, col]: pair = q*NQP + jj consumes f-tiles
        # (pix=2jj+i2, q); W row for (pix, q, dd) is f = pix*256 + q*128 + dd.
        wc5 = wfull[i * FLOC:(i + 1) * FLOC].reshape(NQP, 2, 2, 128, KOUT)
        wl = np.ascontiguousarray(wc5.transpose(3, 2, 0, 1, 4)) \
            .reshape(128, NPAIR, 2, KOUT)

        ft4 = featT_i.reshape(4, 128, NB)
        in_maps.append({
            "fxh": np.ascontiguousarray(
                np.concatenate([convw_dev, ft4[0][:, :1024]], axis=1)),
            "fxt": np.ascontiguousarray(ft4[0][:, 1024:]),
            "featT": np.ascontiguousarray(ft4[1:]),
            "convb": convb_dev,
            "wmat": wl,
        })
    return in_maps


def _run_device(in_maps, trace=False, **kw):
    from concourse.bass_utils import run_bass_kernel_spmd

    if "nc" not in _STATE:
        _STATE["nc"] = _build_module()
    nc = _STATE["nc"]
    return run_bass_kernel_spmd(
        nc, in_maps, core_ids=list(range(NCORES)), trace=trace, **kw
    )


def _postprocess(partial, roi, cls_b, bbox_b):
    f32 = np.float32
    part = partial.astype(f32) / f32(S_H * S_W)
    cls_b = np.asarray(cls_b, dtype=f32)
    bbox_b = np.asarray(bbox_b, dtype=f32)
    clsdiff = part[:, :P] + (cls_b[P:] - cls_b[:P])
    bb1 = part[:, P:2 * P] + bbox_b[P:2 * P]
    bb2 = part[:, 2 * P:3 * P] + bbox_b[2 * P:3 * P]
    bb3 = part[:, 3 * P:] + bbox_b[3 * P:]

    obj = 1.0 / (1.0 + np.exp(-clsdiff, dtype=f32))
    roi_img = roi.astype(f32) * f32(STRIDE)
    x = roi_img[:, :, 0] - bb1 * roi_img[:, :, 3]
    y = roi_img[:, :, 1]
    w = roi_img[:, :, 2] * np.exp(np.clip(bb2, -10.0, 10.0), dtype=f32)
    hh = roi_img[:, :, 3] * np.exp(np.clip(bb3, -10.0, 10.0), dtype=f32)
    return np.stack([x, y, w, hh, obj], axis=-1).astype(f32)


def kernel(features, roi, conv_w, conv_b, cls_w, cls_b, bbox_w, bbox_b):
    in_maps = _prep_inputs(features, conv_w, conv_b, cls_w, bbox_w)
    res = _run_device(in_maps)
    partial = np.zeros((B, KOUT), dtype=np.float64)
    for r in res.results:
        partial += np.asarray(r["out"], dtype=np.float64)
    return _postprocess(partial.astype(np.float32), np.asarray(roi),
                        np.asarray(cls_b), np.asarray(bbox_b))


# revision 24
# speedup vs baseline: 2.4047x; 1.0028x over previous
"""Trainium2 Bass kernel for nn_ClassifierModel (nms_detection).

Computation (reference):
    h    = relu(features @ conv_w + conv_b)        # (B,H,W,C)@(C,D) -> (B,H,W,D)
    flat = h.reshape(B, F)                         # F = H*W*D = 401408
    cls  = flat @ cls_w + cls_b                    # (B, 64)
    bbox = flat @ bbox_w + bbox_b                  # (B, 128)
    <tiny postprocessing with roi -> (B, P, 5)>

The reference never uses bbox coordinate 0 (x is overwritten by the second
assignment) and objectness only depends on cls1-cls0, so the device only
computes 128 output columns: [cls1-cls0 | bb1 | bb2 | bb3].

Sharding: the flatten (contraction) dim F is split across the 8 cores by
slicing H into 8 chunks of 28 rows. Each core computes its conv slice and a
partial (B, 128) product against its slice of the fused weight matrix; the
host sums the 8 partials and runs the tiny postprocessing. This reads each
dense-weight element exactly once across the machine.

Everything streams in fp8 (the kernel is HBM-bound): features and conv_w in
e3m4 (4 mantissa bits), hT and the fused dense weights in e4m3 so stage 2 can
run MatmulPerfMode.DoubleRow (two 128-deep f-tiles per instruction at 0.5
cycles/row). Host-side scales (S_FEAT/S_H/S_W) keep all values inside the TRN
fp8 normal ranges (e4m3 max normal is +-240) and are divided out on the host.

Per-core device layout:
    featT  (4,128,NB)        : features slice, (c, pix*B+b) columns, e3m4 * S_FEAT
    convw  (128, 4*256)      : conv_w k-tiles (c on partitions), e3m4 * S_H/S_FEAT
    convb  (2,128,1)         : conv_b halves (d on partitions), fp32 * S_H
    wmat   (128,NPAIR,2,128) : fused W rows, DoubleRow pair-major, e4m3 * S_W
    out    (16,128)          : partial fp32 sums for this core's f range

Stage 1 produces h^T with d on partitions and (pix, b) on the free axis; a
128-partition f-tile of flat^T is exactly hT[q][:, pix, :], and a DoubleRow
pair (pix=2j, 2j+1) is the contiguous 32-column slice hT[q][:, 2j:2j+2, :].
Stage 2 consumes pairs in q-major order (all q=0 pairs, then q=1) so it can
start as soon as hT[0] exists; the host lays wmat out in the same order.
"""

import numpy as np

B = 16
H, W, C = 224, 7, 512
D = 256
P = 32
NCORES = 8
HSH = H // NCORES          # 28 rows of H per core
PIX = HSH * W              # 196 pixels per core per batch
FLOC = PIX * D             # 50176 contraction elements per core
NB = PIX * B               # 3136 stage-1 moving columns
NT = FLOC // 128           # 392 f-tiles per core
NPAIR = NT // 2            # 196 DoubleRow pairs per core
NQP = NPAIR // 2           # 98 pairs per d-half
KOUT = 128                 # device output columns [clsdiff|bb1|bb2|bb3]
NTILE = 448                # stage-1 moving tile (3136 = 7*448)
NTI = NB // NTILE          # 7 stage-1 n-tiles
PCHUNKS = [21] * 8 + [14, 7, 4, 2, 1]   # W-stream chunks in pairs (sum = 196)
STRIDE = 16.0
S_FEAT = 2.0               # features scale into e3m4
S_H = 32.0                 # hT scale into e4m3
S_W = 16384.0              # dense-weight scale into e4m3

_STATE = {}


def _build_module(reps=1):
    import concourse.mybir as mybir
    import concourse.tile as tile
    from concourse import bacc

    f32 = mybir.dt.float32
    f8e3 = mybir.dt.float8e3
    f8e4 = mybir.dt.float8e4
    nc = bacc.Bacc("TRN2", target_bir_lowering=False, debug=False)

    # Head split: fxh = conv_w + first 1024 cols of featT[0] (728ns transfer
    # hides the next DMA's HWDGE+DGE issue latency -> no stream bubble);
    # fxt = remaining 2112 cols of featT[0].
    fxh = nc.dram_tensor("fxh", [128, 4 * D + 1024], f8e3,
                         kind="ExternalInput")
    fxt = nc.dram_tensor("fxt", [128, NB - 1024], f8e3, kind="ExternalInput")
    featT = nc.dram_tensor("featT", [3, 128, NB], f8e3, kind="ExternalInput")
    convb = nc.dram_tensor("convb", [2, 128, 1], f32, kind="ExternalInput")
    wmat = nc.dram_tensor("wmat", [128, NPAIR, 2, KOUT], f8e4,
                          kind="ExternalInput")
    if reps == 1:
        out = nc.dram_tensor("out", [KOUT, B], f32, kind="ExternalOutput")
    else:
        out = nc.dram_tensor("out", [reps, KOUT, B], f32,
                             kind="ExternalOutput")

    with tile.TileContext(nc) as tc:
        with (
            tc.tile_pool(name="res", bufs=2 if reps > 1 else 1) as res,
            tc.tile_pool(name="win", bufs=8) as win,
            tc.tile_pool(name="ps1", bufs=7, space="PSUM") as ps1p,
            tc.tile_pool(name="ps2", bufs=1, space="PSUM") as ps2p,
        ):
            for rep in range(reps):
                # DMA order: conv_w first — its Ldweights issues ~1.4us before
                # the first matmul and anchors the PE p-state ramp (measured:
                # any ordering that delays the first PE instruction leaves
                # stage 1 at the MID/LOW p-state, which finishes hts[1] late
                # and stalls the W stream on the win-pool window). Then the
                # four big featT transfers (HWDGE gen pipelines behind them),
                # tiny biases last (needed only by the activations ~7us in).
                big = res.tile([128, 4 * D + NB], f8e3, tag="big", name="big")
                nc.sync.dma_start(big[:, :4 * D + 1024], fxh[:])
                nc.sync.dma_start(big[:, 4 * D + 1024:], fxt[:])
                cw = big[:, :4 * D]
                xts = [big[:, 4 * D:]]
                for t in range(3):
                    xt = res.tile([128, NB], f8e3, tag=f"xt{t}", name=f"xt{t}")
                    nc.sync.dma_start(xt[:], featT[t])
                    xts.append(xt)
                cbs = []
                for q in range(2):
                    cb = res.tile([128, 1], f32, tag=f"cb{q}", name=f"cb{q}")
                    nc.sync.dma_start(cb[:], convb[q])
                    cbs.append(cb)
                hts = [res.tile([128, PIX, B], f8e4, tag=f"ht{q}",
                                name=f"ht{q}") for q in range(2)]

                # Stage 1, k-outer so matmuls start as soon as xt[0] lands:
                # hT[q][:, n-tile] = relu(conv_w[:, q-half].T @ featT + b)
                for q in range(2):
                    pss = [ps1p.tile([128, NTILE], f32, tag="ps",
                                     name=f"ps{q}_{n}") for n in range(NTI)]
                    # k=0 runs n=6 first: its first matmul depends on the fxt
                    # DMA, landing ~750ns after the Ldweights (cw via fxh) so
                    # the PE p-state ramp is anchored early.
                    for k in range(4):
                        for n in ([6, 0, 1, 2, 3, 4, 5] if k == 0
                                  else range(NTI)):
                            nc.tensor.matmul(
                                pss[n][:],
                                cw[:, k * D + q * 128:k * D + (q + 1) * 128],
                                xts[k][:, n * NTILE:(n + 1) * NTILE],
                                start=(k == 0),
                                stop=(k == 3),
                            )
                    for n in range(NTI):
                        nc.scalar.activation(
                            hts[q][:, n * HSH:(n + 1) * HSH, :],
                            pss[n][:],
                            mybir.ActivationFunctionType.Relu,
                            bias=cbs[q],
                        )

                # Stage 2: acc(128,16) += DoubleRow pair matmuls, W stationary
                # and hT moving: out free is 16, so each pair costs only 8 PE
                # cycles, and the tail copy/DMA handle a (128,16) tile.
                # pair p: lhsT = wc[:, t]              (128,2,128)
                #         rhs  = hT[q][:, 2j:2j+2, :]  (128,2,16)
                acc = ps2p.tile([KOUT, B], f32, tag="acc", name="acc")
                pos = 0
                for ch in PCHUNKS:
                    wc = win.tile([128, ch, 2, KOUT], f8e4, tag="wc",
                                  name="wc")
                    nc.sync.dma_start(wc[:], wmat[:, pos:pos + ch])
                    for t in range(ch):
                        p_ = pos + t
                        q, jj = (0, p_) if p_ < NQP else (1, p_ - NQP)
                        nc.tensor.matmul(
                            acc[:],
                            wc[:, t],
                            hts[q][:, 2 * jj:2 * jj + 2, :],
                            start=(p_ == 0),
                            stop=(p_ == NPAIR - 1),
                            perf_mode=mybir.MatmulPerfMode.DoubleRow,
                        )
                    pos += ch

                ot = res.tile([KOUT, B], f32, tag="ot", name="ot")
                nc.vector.tensor_copy(ot[:], acc[:])
                nc.sync.dma_start(out[:] if reps == 1 else out[rep], ot[:])

    nc.compile()
    return nc


def _prep_inputs(features, conv_w, conv_b, cls_w, bbox_w):
    import ml_dtypes

    f32 = np.float32
    f8e3 = ml_dtypes.float8_e3m4
    f8e4 = ml_dtypes.float8_e4m3

    features = (np.asarray(features, dtype=f32) * f32(S_FEAT)).astype(f8e3)
    convw_dev = np.ascontiguousarray(
        (np.asarray(conv_w, dtype=f32) * f32(S_H / S_FEAT))
        .reshape(4, 128, D).transpose(1, 0, 2).reshape(128, 4 * D)
    ).astype(f8e3)
    convb_dev = (np.ascontiguousarray(conv_b, dtype=f32) * f32(S_H)) \
        .reshape(2, 128, 1)

    # Fused device weights: [cls1-cls0 | bb1 | bb2 | bb3]  (F, 128)
    cls_w = np.asarray(cls_w, dtype=f32)
    bbox_w = np.asarray(bbox_w, dtype=f32)
    wfull = np.concatenate(
        [cls_w[:, P:] - cls_w[:, :P], bbox_w[:, P:]], axis=1) * f32(S_W)
    np.clip(wfull, -240.0, 240.0, out=wfull)
    wfull = wfull.astype(f8e4)

    in_maps = []
    for i in range(NCORES):
        fi = features[:, i * HSH:(i + 1) * HSH, :, :].reshape(B, PIX, C)
        featT_i = np.ascontiguousarray(fi.transpose(2, 1, 0).reshape(C, NB))

        # wmat[dd, pair, i2, col]: pair = q*NQP + jj consumes f-tiles
        # (pix=2jj+i2, q); W row for (pix, q, dd) is f = pix*256 + q*128 + dd.
        wc5 = wfull[i * FLOC:(i + 1) * FLOC].reshape(NQP, 2, 2, 128, KOUT)
        wl = np.ascontiguousarray(wc5.transpose(3, 2, 0, 1, 4)) \
            .reshape(128, NPAIR, 2, KOUT)

        ft4 = featT_i.reshape(4, 128, NB)
        in_maps.append({
            "fxh": np.ascontiguousarray(
                np.concatenate([convw_dev, ft4[0][:, :1024]], axis=1)),
            "fxt": np.ascontiguousarray(ft4[0][:, 1024:]),
            "featT": np.ascontiguousarray(ft4[1:]),
            "convb": convb_dev,
            "wmat": wl,
        })
    return in_maps


def _run_device(in_maps, trace=False, **kw):
    from concourse.bass_utils import run_bass_kernel_spmd

    if "nc" not in _STATE:
        _STATE["nc"] = _build_module()
    nc = _STATE["nc"]
    return run_bass_kernel_spmd(
        nc, in_maps, core_ids=list(range(NCORES)), trace=trace, **kw
    )


def _postprocess(partial, roi, cls_b, bbox_b):
    f32 = np.float32
    part = partial.astype(f32) / f32(S_H * S_W)
    cls_b = np.asarray(cls_b, dtype=f32)
    bbox_b = np.asarray(bbox_b, dtype=f32)
    clsdiff = part[:, :P] + (cls_b[P:] - cls_b[:P])
    bb1 = part[:, P:2 * P] + bbox_b[P:2 * P]
    bb2 = part[:, 2 * P:3 * P] + bbox_b[2 * P:3 * P]
    bb3 = part[:, 3 * P:] + bbox_b[3 * P:]

    obj = 1.0 / (1.0 + np.exp(-clsdiff, dtype=f32))
    roi_img = roi.astype(f32) * f32(STRIDE)
    x = roi_img[:, :, 0] - bb1 * roi_img[:, :, 3]
    y = roi_img[:, :, 1]
    w = roi_img[:, :, 2] * np.exp(np.clip(bb2, -10.0, 10.0), dtype=f32)
    hh = roi_img[:, :, 3] * np.exp(np.clip(bb3, -10.0, 10.0), dtype=f32)
    return np.stack([x, y, w, hh, obj], axis=-1).astype(f32)


def kernel(features, roi, conv_w, conv_b, cls_w, cls_b, bbox_w, bbox_b):
    in_maps = _prep_inputs(features, conv_w, conv_b, cls_w, bbox_w)
    res = _run_device(in_maps)
    partial = np.zeros((KOUT, B), dtype=np.float64)
    for r in res.results:
        partial += np.asarray(r["out"], dtype=np.float64)
    partial = partial.T
    return _postprocess(partial.astype(np.float32), np.asarray(roi),
                        np.asarray(cls_b), np.asarray(bbox_b))


# revision 32
# speedup vs baseline: 2.4099x; 1.0021x over previous
"""Trainium2 Bass kernel for nn_ClassifierModel (nms_detection).

Computation (reference):
    h    = relu(features @ conv_w + conv_b)        # (B,H,W,C)@(C,D) -> (B,H,W,D)
    flat = h.reshape(B, F)                         # F = H*W*D = 401408
    cls  = flat @ cls_w + cls_b                    # (B, 64)
    bbox = flat @ bbox_w + bbox_b                  # (B, 128)
    <tiny postprocessing with roi -> (B, P, 5)>

The reference never uses bbox coordinate 0 (x is overwritten by the second
assignment) and objectness only depends on cls1-cls0, so the device only
computes 128 output columns: [cls1-cls0 | bb1 | bb2 | bb3].

Sharding: the flatten (contraction) dim F is split across the 8 cores by
slicing H into 8 chunks of 28 rows. Each core computes its conv slice and a
partial (B, 128) product against its slice of the fused weight matrix; the
host sums the 8 partials and runs the tiny postprocessing. This reads each
dense-weight element exactly once across the machine.

Everything streams in fp8 (the kernel is HBM-bound): features and conv_w in
e3m4 (4 mantissa bits), hT and the fused dense weights in e4m3 so stage 2 can
run MatmulPerfMode.DoubleRow (two 128-deep f-tiles per instruction at 0.5
cycles/row). Host-side scales (S_FEAT/S_H/S_W) keep all values inside the TRN
fp8 normal ranges (e4m3 max normal is +-240) and are divided out on the host.

Per-core device layout:
    featT  (4,128,NB)        : features slice, (c, pix*B+b) columns, e3m4 * S_FEAT
    convw  (128, 4*256)      : conv_w k-tiles (c on partitions), e3m4 * S_H/S_FEAT
    convb  (2,128,1)         : conv_b halves (d on partitions), fp32 * S_H
    wmat   (128,NPAIR,2,128) : fused W rows, DoubleRow pair-major, e4m3 * S_W
    out    (16,128)          : partial fp32 sums for this core's f range

Stage 1 produces h^T with d on partitions and (pix, b) on the free axis; a
128-partition f-tile of flat^T is exactly hT[q][:, pix, :], and a DoubleRow
pair (pix=2j, 2j+1) is the contiguous 32-column slice hT[q][:, 2j:2j+2, :].
Stage 2 consumes pairs in q-major order (all q=0 pairs, then q=1) so it can
start as soon as hT[0] exists; the host lays wmat out in the same order.
"""

import numpy as np

B = 16
H, W, C = 224, 7, 512
D = 256
P = 32
NCORES = 8
HSH = H // NCORES          # 28 rows of H per core
PIX = HSH * W              # 196 pixels per core per batch
FLOC = PIX * D             # 50176 contraction elements per core
NB = PIX * B               # 3136 stage-1 moving columns
NT = FLOC // 128           # 392 f-tiles per core
NPAIR = NT // 2            # 196 DoubleRow pairs per core
NQP = NPAIR // 2           # 98 pairs per d-half
KOUT = 128                 # device output columns [clsdiff|bb1|bb2|bb3]
NTILE = 448                # stage-1 moving tile (3136 = 7*448)
NTI = NB // NTILE          # 7 stage-1 n-tiles
PCHUNKS = [21] * 8 + [14, 7, 4, 2, 1]   # W-stream chunks in pairs (sum = 196)
STRIDE = 16.0
S_FEAT = 2.0               # features scale into e3m4
S_H = 32.0                 # hT scale into e4m3
S_W = 16384.0              # dense-weight scale into e4m3

_STATE = {}


def _build_module(reps=1, zero_bias=False):
    import concourse.mybir as mybir
    import concourse.tile as tile
    from concourse import bacc

    f32 = mybir.dt.float32
    f8e3 = mybir.dt.float8e3
    f8e4 = mybir.dt.float8e4
    nc = bacc.Bacc("TRN2", target_bir_lowering=False, debug=False)

    # Head split: fxh = conv_w + first 1024 cols of featT[0] (728ns transfer
    # hides the next DMA's HWDGE+DGE issue latency -> no stream bubble);
    # fxt = remaining 2112 cols of featT[0].
    fxh = nc.dram_tensor("fxh", [128, 4 * D + 1024], f8e3,
                         kind="ExternalInput")
    fxt = nc.dram_tensor("fxt", [128, NB - 1024], f8e3, kind="ExternalInput")
    featT = nc.dram_tensor("featT", [3, 128, NB], f8e3, kind="ExternalInput")
    convb = None if zero_bias else nc.dram_tensor(
        "convb", [2, 128, 1], f32, kind="ExternalInput")
    wmat = nc.dram_tensor("wmat", [128, NPAIR, 2, KOUT], f8e4,
                          kind="ExternalInput")
    if reps == 1:
        out = nc.dram_tensor("out", [KOUT, B], f32, kind="ExternalOutput")
    else:
        out = nc.dram_tensor("out", [reps, KOUT, B], f32,
                             kind="ExternalOutput")

    with tile.TileContext(nc) as tc:
        with (
            tc.tile_pool(name="res", bufs=2 if reps > 1 else 1) as res,
            tc.tile_pool(name="win", bufs=8) as win,
            tc.tile_pool(name="ps1", bufs=7, space="PSUM") as ps1p,
            tc.tile_pool(name="ps2", bufs=1, space="PSUM") as ps2p,
        ):
            for rep in range(reps):
                # DMA order: conv_w first — its Ldweights issues ~1.4us before
                # the first matmul and anchors the PE p-state ramp (measured:
                # any ordering that delays the first PE instruction leaves
                # stage 1 at the MID/LOW p-state, which finishes hts[1] late
                # and stalls the W stream on the win-pool window). Then the
                # four big featT transfers (HWDGE gen pipelines behind them),
                # tiny biases last (needed only by the activations ~7us in).
                big = res.tile([128, 4 * D + NB], f8e3, tag="big", name="big")
                nc.sync.dma_start(big[:, :4 * D + 1024], fxh[:])
                nc.sync.dma_start(big[:, 4 * D + 1024:], fxt[:])
                cw = big[:, :4 * D]
                xts = [big[:, 4 * D:]]
                for t in range(3):
                    xt = res.tile([128, NB], f8e3, tag=f"xt{t}", name=f"xt{t}")
                    nc.sync.dma_start(xt[:], featT[t])
                    xts.append(xt)
                cbs = []
                if not zero_bias:
                    for q in range(2):
                        cb = res.tile([128, 1], f32, tag=f"cb{q}",
                                      name=f"cb{q}")
                        nc.sync.dma_start(cb[:], convb[q])
                        cbs.append(cb)
                hts = [res.tile([128, PIX, B], f8e4, tag=f"ht{q}",
                                name=f"ht{q}") for q in range(2)]

                # Stage 1, k-outer so matmuls start as soon as xt[0] lands:
                # hT[q][:, n-tile] = relu(conv_w[:, q-half].T @ featT + b)
                for q in range(2):
                    pss = [ps1p.tile([128, NTILE], f32, tag="ps",
                                     name=f"ps{q}_{n}") for n in range(NTI)]
                    # k=0 runs n=6 first: its first matmul depends on the fxt
                    # DMA, landing ~750ns after the Ldweights (cw via fxh) so
                    # the PE p-state ramp is anchored early.
                    for k in range(4):
                        for n in ([6, 0, 1, 2, 3, 4, 5] if k == 0
                                  else range(NTI)):
                            nc.tensor.matmul(
                                pss[n][:],
                                cw[:, k * D + q * 128:k * D + (q + 1) * 128],
                                xts[k][:, n * NTILE:(n + 1) * NTILE],
                                start=(k == 0),
                                stop=(k == 3),
                            )
                    for n in range(NTI):
                        nc.scalar.activation(
                            hts[q][:, n * HSH:(n + 1) * HSH, :],
                            pss[n][:],
                            mybir.ActivationFunctionType.Relu,
                            bias=0.0 if zero_bias else cbs[q],
                        )

                # Stage 2: acc(128,16) += DoubleRow pair matmuls, W stationary
                # and hT moving: out free is 16, so each pair costs only 8 PE
                # cycles, and the tail copy/DMA handle a (128,16) tile.
                # pair p: lhsT = wc[:, t]              (128,2,128)
                #         rhs  = hT[q][:, 2j:2j+2, :]  (128,2,16)
                acc = ps2p.tile([KOUT, B], f32, tag="acc", name="acc")
                pos = 0
                for ch in PCHUNKS:
                    wc = win.tile([128, ch, 2, KOUT], f8e4, tag="wc",
                                  name="wc")
                    nc.sync.dma_start(wc[:], wmat[:, pos:pos + ch])
                    for t in range(ch):
                        p_ = pos + t
                        q, jj = (0, p_) if p_ < NQP else (1, p_ - NQP)
                        nc.tensor.matmul(
                            acc[:],
                            wc[:, t],
                            hts[q][:, 2 * jj:2 * jj + 2, :],
                            start=(p_ == 0),
                            stop=(p_ == NPAIR - 1),
                            perf_mode=mybir.MatmulPerfMode.DoubleRow,
                        )
                    pos += ch

                ot = res.tile([KOUT, B], f32, tag="ot", name="ot")
                nc.vector.tensor_copy(ot[:], acc[:])
                nc.sync.dma_start(out[:] if reps == 1 else out[rep], ot[:])

    nc.compile()
    return nc


def _prep_inputs(features, conv_w, conv_b, cls_w, bbox_w, zero_bias=False):
    import ml_dtypes

    f32 = np.float32
    f8e3 = ml_dtypes.float8_e3m4
    f8e4 = ml_dtypes.float8_e4m3

    features = (np.asarray(features, dtype=f32) * f32(S_FEAT)).astype(f8e3)
    convw_dev = np.ascontiguousarray(
        (np.asarray(conv_w, dtype=f32) * f32(S_H / S_FEAT))
        .reshape(4, 128, D).transpose(1, 0, 2).reshape(128, 4 * D)
    ).astype(f8e3)
    convb_dev = (np.ascontiguousarray(conv_b, dtype=f32) * f32(S_H)) \
        .reshape(2, 128, 1)

    # Fused device weights: [cls1-cls0 | bb1 | bb2 | bb3]  (F, 128)
    cls_w = np.asarray(cls_w, dtype=f32)
    bbox_w = np.asarray(bbox_w, dtype=f32)
    wfull = np.concatenate(
        [cls_w[:, P:] - cls_w[:, :P], bbox_w[:, P:]], axis=1) * f32(S_W)
    np.clip(wfull, -240.0, 240.0, out=wfull)
    wfull = wfull.astype(f8e4)

    in_maps = []
    for i in range(NCORES):
        fi = features[:, i * HSH:(i + 1) * HSH, :, :].reshape(B, PIX, C)
        featT_i = np.ascontiguousarray(fi.transpose(2, 1, 0).reshape(C, NB))

        # wmat[dd, pair, i2, col]: pair = q*NQP + jj consumes f-tiles
        # (pix=2jj+i2, q); W row for (pix, q, dd) is f = pix*256 + q*128 + dd.
        wc5 = wfull[i * FLOC:(i + 1) * FLOC].reshape(NQP, 2, 2, 128, KOUT)
        wl = np.ascontiguousarray(wc5.transpose(3, 2, 0, 1, 4)) \
            .reshape(128, NPAIR, 2, KOUT)

        ft4 = featT_i.reshape(4, 128, NB)
        im = {
            "fxh": np.ascontiguousarray(
                np.concatenate([convw_dev, ft4[0][:, :1024]], axis=1)),
            "fxt": np.ascontiguousarray(ft4[0][:, 1024:]),
            "featT": np.ascontiguousarray(ft4[1:]),
            "wmat": wl,
        }
        if not zero_bias:
            im["convb"] = convb_dev
        in_maps.append(im)
    return in_maps


def _run_device(in_maps, trace=False, zero_bias=False, **kw):
    from concourse.bass_utils import run_bass_kernel_spmd

    key = "nc_zb" if zero_bias else "nc"
    if key not in _STATE:
        _STATE[key] = _build_module(zero_bias=zero_bias)
        _STATE["nc"] = _STATE[key]   # latest module, for test.py's TimelineSim
    nc = _STATE[key]
    return run_bass_kernel_spmd(
        nc, in_maps, core_ids=list(range(NCORES)), trace=trace, **kw
    )


def _postprocess(partial, roi, cls_b, bbox_b):
    f32 = np.float32
    part = partial.astype(f32) / f32(S_H * S_W)
    cls_b = np.asarray(cls_b, dtype=f32)
    bbox_b = np.asarray(bbox_b, dtype=f32)
    clsdiff = part[:, :P] + (cls_b[P:] - cls_b[:P])
    bb1 = part[:, P:2 * P] + bbox_b[P:2 * P]
    bb2 = part[:, 2 * P:3 * P] + bbox_b[2 * P:3 * P]
    bb3 = part[:, 3 * P:] + bbox_b[3 * P:]

    obj = 1.0 / (1.0 + np.exp(-clsdiff, dtype=f32))
    roi_img = roi.astype(f32) * f32(STRIDE)
    x = roi_img[:, :, 0] - bb1 * roi_img[:, :, 3]
    y = roi_img[:, :, 1]
    w = roi_img[:, :, 2] * np.exp(np.clip(bb2, -10.0, 10.0), dtype=f32)
    hh = roi_img[:, :, 3] * np.exp(np.clip(bb3, -10.0, 10.0), dtype=f32)
    return np.stack([x, y, w, hh, obj], axis=-1).astype(f32)


def kernel(features, roi, conv_w, conv_b, cls_w, cls_b, bbox_w, bbox_b):
    zb = not np.any(np.asarray(conv_b))
    in_maps = _prep_inputs(features, conv_w, conv_b, cls_w, bbox_w,
                           zero_bias=zb)
    res = _run_device(in_maps, zero_bias=zb)
    partial = np.zeros((KOUT, B), dtype=np.float64)
    for r in res.results:
        partial += np.asarray(r["out"], dtype=np.float64)
    partial = partial.T
    return _postprocess(partial.astype(np.float32), np.asarray(roi),
                        np.asarray(cls_b), np.asarray(bbox_b))
